# revision 42
# baseline (speedup 1.0000x reference)
"""GCN (3-layer, skip-concat) on 8 Trainium2 NeuronCores.

Strategy (hardcoded for N=10000, E=320000, dims 512/512/256):
  - Row-partition nodes across 8 cores (1280 padded rows each, N padded
    to 10240). The adjacency shard is densified on the host into
    A_k^T [10240, 1280] and stored twice in fp8: e4m3 for layers 0/1
    (whose SpMM runs fp8 DoubleRow) and e3m4 for the final layer.
    fp8 halves adjacency HBM traffic vs bf16.
  - Layer-0 dense (x @ W0) also runs DoubleRow: x, W0 in e4m3 with W0
    host-scaled by 16 (clears the e4m3 denormal floor), epilogue x1/16.
  - Layers 0/1 SpMM: S blocks (e4m3) stationary + A^T (e4m3) moving with
    perf_mode=DoubleRow -> 256-deep contraction per instruction, ~2x
    TensorE throughput. Layer 2 SpMM: e3m4 S (host-scale 1/8, sink x8)
    x e3m4 A, plain matmuls. End-to-end rel err ~1.3e-2 (verified
    numerically on the fixed seed; gate is 2e-2).
  - Activations live feature-major (X^T) in SBUF. Per layer:
      S_k   = X^T.T @ W          (node-major [1280, d_out], PE)
      S     = AllGather(S_k)     (fp8, HBM collective, 3 chunks)
      Y^T   = S_tiles^T @ A_k^T  (feature-major, PE; S tiles stationary)
      X' ^T = [relu(Y^T + b); (Y^T + b)]   (partition-axis concat, free)
  - Chunking: 3 gathered chunks per layer = local row ranges
    [0,512) / [512,1024) / [1024,1280), produced by passes nci0/1/2.
    Natural pass order (0,1,2): the first pass of each layer is a long
    512-wide one, so the last-produced chunk of the next layer (small
    ch2, consumed last in the contraction) has maximal all-gather slack.
  - A tiny warm-up AllGather at program start absorbs the CC-stream
    bootstrap (~35-110us) + initial cross-core skew, so the first real
    gather never eats them.
  - Queue discipline: sync = x packs + adjacency prefetch + deferred
    W1/W2/bias loads (trickled); scalar = epilogue drains (alternating
    with vector), bounce writes, gathered-chunk reloads; gpsimd = w0 +
    collective triggers. Layer-1's s_ch reloads are emitted mid-k-loop
    (chunk hooks) so they start the moment their WAR hazard clears.
  - SpMM PSUM tiles rotate through 5 tags so the first matmul of each
    pass never waits on the previous pass's epilogue draining the bank
    (5 spmm banks + 2 dense banks = 7 of 8).
"""

import os
import numpy as np
from ml_dtypes import bfloat16, float8_e3m4, float8_e4m3

N = 10000
NPAD = 10240
NCORES = 8
R = NPAD // NCORES  # 1280 rows per core
P = 128
CT = NPAD // P  # 80 contraction tiles for the SpMM
KSUB = 4  # contraction subtiles per adjacency DMA block

D0_IN, D0_OUT = 512, 512
D1_IN, D1_OUT = 1024, 512
D2_IN, D2_OUT = 1024, 256

# n-chunks of the 1280-wide free dim (PSUM bank = 512 fp32); pass nci
# produces gathered chunk nci (same index).
N_CHUNKS = [(0, 512), (512, 512), (1024, 256)]
# natural pass order: the FIRST pass of each layer is a long (512-wide)
# one, so the last-produced gathered chunk of the NEXT layer (small ch2,
# consumed last in the contraction) has maximal all-gather slack.
NCI_ORDER = (0, 1, 2)
# chunk c -> (local row0, nrows, gathered ktiles, KSUB-groups)
CHUNKS = {0: (0, 512, 32, 8), 1: (512, 512, 32, 8), 2: (1024, 256, 16, 4)}
C_ORDER = (0, 1, 2)  # contraction + production order
CH_BASE = {0: 0, 1: 4096, 2: 8192}  # gathered-row base of each chunk
# flat adjacency block order: (chunk, group) pairs in contraction order
BLOCKS = [(c, g) for c in C_ORDER for g in range(CHUNKS[c][3])]  # 20
# dense m-tiles whose lhsT columns come from pass nci's epilogue
M_OF_NC = {0: (0, 1, 2, 3), 1: (4, 5, 6, 7), 2: (8, 9)}

SHARD_L0 = bool(int(os.environ.get("GCN_SHARD_L0", "0")))

_CACHE = {}
LAST_RESULT = None  # BassKernelResults of the most recent run (for test.py)


def _build_bass():
    import concourse.bass as bass
    import concourse.bacc as bacc
    import concourse.mybir as mybir
    import concourse.tile as tile

    dt = mybir.dt
    bf16 = dt.bfloat16
    e4 = dt.float8e4
    e3 = dt.float8e3
    f32 = dt.float32
    ts = bass.ts
    DR = mybir.MatmulPerfMode.DoubleRow

    nc = bacc.Bacc(
        "TRN2",
        target_bir_lowering=False,
        debug=False,
        enable_asserts=False,
        num_devices=NCORES,
    )

    # x and W0 are e4m3: the layer-0 dense runs fp8 DoubleRow. W0 is
    # host-scaled by 16 (so its small values clear the e4m3 denormal
    # floor); the epilogue multiplies by 1/16. x tiles are packed 4
    # m-tiles per DMA so the dense loop isn't DMA-issue-rate bound.
    n_xp = 5 if SHARD_L0 else CT // 4
    xp_w = 256 if SHARD_L0 else 512
    xTf_d = nc.dram_tensor("xTf", [n_xp, P, D0_IN // P, xp_w], e4, kind="ExternalInput")
    # adjacency blocks [P, KSUB, nw]: e4m3 copy (layers 0/1, DoubleRow)
    # and e3m4 copy (layer 2); nci0/1 are the 512-wide column chunks.
    adjA01_d = nc.dram_tensor("adjA01", [2, 20, P, KSUB, 512], e4, kind="ExternalInput")
    adjA2_d = nc.dram_tensor("adjA2", [20, P, KSUB, 256], e4, kind="ExternalInput")
    adjB01_d = nc.dram_tensor("adjB01", [2, 20, P, KSUB, 512], e3, kind="ExternalInput")
    adjB2_d = nc.dram_tensor("adjB2", [20, P, KSUB, 256], e3, kind="ExternalInput")
    W_d = [
        nc.dram_tensor("W0", [D0_IN, D0_OUT], e4, kind="ExternalInput"),
        nc.dram_tensor("W1", [D1_IN, D1_OUT], bf16, kind="ExternalInput"),
        nc.dram_tensor("W2", [D2_IN, D2_OUT], bf16, kind="ExternalInput"),
    ]
    b_d = [
        nc.dram_tensor("b0", [D0_OUT, 1], f32, kind="ExternalInput"),
        nc.dram_tensor("b1", [D1_OUT, 1], f32, kind="ExternalInput"),
        nc.dram_tensor("b2", [D2_OUT, 1], f32, kind="ExternalInput"),
    ]
    outT_d = nc.dram_tensor("outT", [D2_OUT, R], f32, kind="ExternalOutput")

    DIMS = [(D0_IN, D0_OUT), (D1_IN, D1_OUT), (D2_IN, D2_OUT)]
    # per-layer S dtype on the gather path (stationary dtype of the
    # CONSUMING spmm): layers 0/1 e4m3 (DoubleRow), layer 2 bf16.
    S_DT = [e4, e4, e3]

    with tile.TileContext(nc) as tc:
        ctx_pools = (
            tc.tile_pool(name="persist", bufs=1),
            tc.tile_pool(name="work", bufs=3),
            tc.tile_pool(name="psum", bufs=1, space="PSUM"),
            tc.tile_pool(name="dram", bufs=1, space="DRAM"),
        )
        with ctx_pools[0] as persist, ctx_pools[1] as work, \
             ctx_pools[2] as psum_pool, ctx_pools[3] as dram_pool:

            # ---- resident weights / biases ----
            # w0 is needed immediately but issued on the gpsimd queue so the
            # sync queue's first DMAs are the xtile loads the PE waits on.
            w_sb = []
            for L, (d_in, d_out) in enumerate(DIMS):
                wt = persist.tile(
                    [P, d_in // P, d_out], e4 if L == 0 else bf16,
                    name=f"w{L}", tag=f"w{L}"
                )
                if L == 0:
                    for c in range(d_in // P):
                        nc.gpsimd.dma_start(wt[:, c, :], W_d[L][ts(c, P), :])
                w_sb.append(wt)
            warm_done = [False]

            b_sb = []
            for L, (d_in, d_out) in enumerate(DIMS):
                tiles = []
                for pi in range(d_out // P):
                    bt = persist.tile([P, 1], f32, name=f"b{L}_{pi}", tag=f"b{L}_{pi}")
                    tiles.append(bt)
                b_sb.append(tiles)

            def deferred_weight_dmas():
                """generator of thunks: W1/W2/bias loads, issued a few per
                dense iteration on the sync queue (it has spare issue slots
                during the dense phase; scalar/vector are epilogue-busy)."""
                for L in (1, 2):
                    d_in = DIMS[L][0]
                    for c in range(d_in // P):
                        yield lambda L=L, c=c: nc.sync.dma_start(
                            w_sb[L][:, c, :], W_d[L][ts(c, P), :]
                        )
                for L in range(3):
                    for pi in range(DIMS[L][1] // P):
                        yield lambda L=L, pi=pi: nc.sync.dma_start(
                            b_sb[L][pi][:], b_d[L][ts(pi, P), :]
                        )

            # ---- activations X^T (feature-major), one 3D tile per layer ----
            xt1 = persist.tile([P, D1_IN // P, R], bf16, name="xt1", tag="xt1")
            xt2 = persist.tile([P, D2_IN // P, R], bf16, name="xt2", tag="xt2")
            xts = [None, xt1, xt2]

            # gathered S: e4m3 tiles for layers 0/1 (DoubleRow stationary),
            # bf16 tiles for layer 2.
            s_ch8 = {
                c: persist.tile(
                    [P, CHUNKS[c][2], 512], e4, name=f"s8_{c}", tag=f"s8_{c}"
                )
                for c in range(3)
            }
            # layer 2's gathered S2 is e3m4, host-semantics scale 1/8
            # (values up to ~105 vs e3m4 max 15.5); sink_out multiplies by 8.
            s_ch16 = {
                c: persist.tile(
                    [P, CHUNKS[c][2], 256], e3, name=f"s16_{c}", tag=f"s16_{c}"
                )
                for c in range(3)
            }
            s_ch_of = [s_ch8, s_ch8, s_ch16]

            # tiny warm-up collective: absorbs the CC-stream bootstrap
            # (~35-110us, starts ~21us in) and initial cross-core skew so
            # the first REAL all-gather doesn't eat them. Nobody consumes
            # its output.
            cc_warm_in = dram_pool.tile([8, 8], bf16, name="ccw_in", tag="ccw_in")
            cc_warm_out = dram_pool.tile(
                [8 * NCORES, 8], bf16, name="ccw_out", tag="ccw_out",
                addr_space="Shared",
            )

            s_bounce = [
                dram_pool.tile([R, DIMS[L][1]], S_DT[L], name=f"s_bounce{L}", tag=f"sb{L}")
                for L in range(3)
            ]
            s_all = [
                {
                    c: dram_pool.tile(
                        [NCORES * CHUNKS[c][1], DIMS[L][1]],
                        S_DT[L],
                        name=f"s_all{L}_{c}",
                        tag=f"sa{L}_{c}",
                        addr_space="Shared",
                    )
                    for c in range(3)
                }
                for L in range(3)
            ]

            # rotating spmm psum tags: 5 tags over passes of <=4 tiles each
            sp_ctr = [0]

            def dense_m(L, m):
                """dense S_k m-tile: psum = xt.T @ W, cast to S dtype, bounce."""
                d_in, d_out = DIMS[L]
                n_ct = d_in // P
                dps = psum_pool.tile(
                    [P, d_out], f32, name=f"dps_{L}_{m}", tag="dense_ps", bufs=2
                )
                for c in range(n_ct):
                    nc.tensor.matmul(
                        dps[:],
                        lhsT=xts[L][:, c, ts(m, P)],
                        rhs=w_sb[L][:, c, :],
                        start=(c == 0),
                        stop=(c == n_ct - 1),
                    )
                s_sb = work.tile(
                    [P, d_out], S_DT[L], name=f"ssb_{L}_{m}", tag=f"s_sb{L}", bufs=4
                )
                # alternate the psum->fp8 drain between vector and
                # scalar so neither engine rate-limits the dense bursts
                sc = 0.125 if L == 2 else 1.0
                if m % 2 == 0:
                    if sc == 1.0:
                        nc.vector.tensor_copy(s_sb[:], dps[:])
                    else:
                        nc.vector.tensor_scalar_mul(s_sb[:], dps[:], sc)
                else:
                    nc.scalar.activation(
                        s_sb[:], dps[:], mybir.ActivationFunctionType.Copy,
                        scale=sc,
                    )
                # bounce on the scalar queue so the sync queue's adjacency
                # prefetch is never head-of-line blocked behind it
                nc.scalar.dma_start(s_bounce[L][ts(m, P), :], s_sb[:])

            def cc_warmup():
                if warm_done[0]:
                    return
                warm_done[0] = True
                nc.gpsimd.collective_compute(
                    "AllGather",
                    mybir.AluOpType.bypass,
                    replica_groups=[list(range(NCORES))],
                    ins=[cc_warm_in[:].opt()],
                    outs=[cc_warm_out[:].opt()],
                )

            def ag_issue(L, c):
                """all-gather chunk c of layer L's S (writes s_all only)."""
                r0, nrows, _, _ = CHUNKS[c]
                nc.gpsimd.collective_compute(
                    "AllGather",
                    mybir.AluOpType.bypass,
                    replica_groups=[list(range(NCORES))],
                    ins=[s_bounce[L][r0 : r0 + nrows, :].opt()],
                    outs=[s_all[L][c].opt()],
                )

            def s_load(L, c):
                """load gathered chunk c into SBUF for layer L's spmm, in
                two halves so the first k-tiles land sooner. Issued on the
                scalar queue: the sync queue's tail (adjacency prefetches,
                bounce writes) would head-of-line-block these right at the
                layer boundary."""
                d_out = DIMS[L][1]
                src = s_all[L][c].rearrange("(t p) d -> p t d", p=P)
                dst = s_ch_of[L][c]
                kt = CHUNKS[c][2]
                h = kt // 2
                nc.scalar.dma_start(dst[:, :h, :d_out], src[:, :h, :])
                nc.scalar.dma_start(dst[:, h:, :d_out], src[:, h:, :])

            def spmm_pass_dr(L, nci, sink, chunk_hook=None):
                """DoubleRow SpMM pass (layers 0/1): e4m3 x e4m3.

                chunk_hook(c) is emitted right after chunk c's last block in
                the contraction loop -- used on the layer's final pass to
                emit the next layer's s_ch reloads as early as the WAR
                hazard allows (ahead of this pass's sinks in queue order).
                """
                n_po = DIMS[L][1] // P
                n0, nw = N_CHUNKS[nci]
                sp_ps = []
                for p in range(n_po):
                    tag = sp_ctr[0] % 5
                    sp_ctr[0] += 1
                    sp_ps.append(
                        psum_pool.tile(
                            [P, nw], f32, name=f"sp_{L}_{nci}_{p}", tag=f"sp{tag}"
                        )
                    )
                first = True
                for bi, (c, g) in enumerate(BLOCKS):
                    if nci < 2:
                        at = work.tile(
                            [P, KSUB, 512], e4,
                            name=f"a4_{L}_{nci}_{bi}", tag="at4", bufs=12,
                        )
                        nc.sync.dma_start(at[:], adjA01_d[nci, bi])
                    else:
                        at = work.tile(
                            [P, KSUB, 256], e4,
                            name=f"a4n2_{L}_{bi}", tag="at4n2", bufs=5,
                        )
                        nc.sync.dma_start(at[:], adjA2_d[bi])
                    for sp in range(2):
                        last = (bi == len(BLOCKS) - 1 and sp == 1)
                        kk = g * KSUB + 2 * sp
                        for p in range(n_po):
                            nc.tensor.matmul(
                                sp_ps[p][:],
                                lhsT=s_ch8[c][:, kk : kk + 2, ts(p, P)],
                                rhs=at[:, 2 * sp : 2 * sp + 2, :],
                                start=first,
                                stop=last,
                                perf_mode=DR,
                            )
                        first = False
                    if chunk_hook is not None and g == CHUNKS[c][3] - 1:
                        chunk_hook(c)
                for p in range(n_po):
                    sink(p, sp_ps[p], n0, nw)

            def spmm_pass_l2(nci, sink):
                """Layer-2 SpMM pass: e3m4 S stationary x e3m4 A moving."""
                n_po = DIMS[2][1] // P
                n0, nw = N_CHUNKS[nci]
                sp_ps = []
                for p in range(n_po):
                    tag = sp_ctr[0] % 5
                    sp_ctr[0] += 1
                    sp_ps.append(
                        psum_pool.tile(
                            [P, nw], f32, name=f"sp_2_{nci}_{p}", tag=f"sp{tag}"
                        )
                    )
                first = True
                for bi, (c, g) in enumerate(BLOCKS):
                    if nci < 2:
                        at = work.tile(
                            [P, KSUB, 512], e3,
                            name=f"a3_{nci}_{bi}", tag="at3", bufs=10,
                        )
                        nc.sync.dma_start(at[:], adjB01_d[nci, bi])
                    else:
                        at = work.tile(
                            [P, KSUB, 256], e3,
                            name=f"a3n2_{bi}", tag="at3n2", bufs=5,
                        )
                        nc.sync.dma_start(at[:], adjB2_d[bi])
                    for s in range(KSUB):
                        last = (bi == len(BLOCKS) - 1 and s == KSUB - 1)
                        for p in range(n_po):
                            nc.tensor.matmul(
                                sp_ps[p][:],
                                lhsT=s_ch16[c][:, g * KSUB + s, ts(p, P)],
                                rhs=at[:, s, :],
                                start=first,
                                stop=last,
                            )
                        first = False
                for p in range(n_po):
                    sink(p, sp_ps[p], n0, nw)

            def sink_mid(L):
                n_po = DIMS[L][1] // P

                def sink(p, ps, n0, nw):
                    nc.scalar.activation(
                        xts[L + 1][:, p, n0 : n0 + nw],
                        ps[:],
                        mybir.ActivationFunctionType.Relu,
                        bias=b_sb[L][p][:],
                    )
                    nc.vector.tensor_scalar_add(
                        xts[L + 1][:, n_po + p, n0 : n0 + nw],
                        ps[:],
                        b_sb[L][p][:],
                    )

                return sink

            def sink_out(p, ps, n0, nw):
                ot = work.tile([P, nw], f32, name=f"ot_{n0}_{p}", tag="ot", bufs=3)
                nc.vector.tensor_scalar(
                    ot[:], ps[:], 8.0, b_sb[2][p][:],
                    mybir.AluOpType.mult, mybir.AluOpType.add,
                )
                nc.scalar.dma_start(outT_d[ts(p, P), n0 : n0 + nw], ot[:])

            # ================= pipeline =================
            cc_warmup()
            if SHARD_L0:
                # layer 0 dense: each core computes only its own 1280 rows,
                # bounced + all-gathered. m-order produces chunk 2 first.
                wgen = deferred_weight_dmas()
                done_after = {3: 0, 7: 1, 9: 2}  # m -> AG chunk complete
                for t in range(5):
                    xtile = work.tile(
                        [P, D0_IN // P, 256], e4, name=f"xtile_{t}", tag="xtile",
                        bufs=6,
                    )
                    nc.sync.dma_start(xtile[:], xTf_d[t])
                    for mm in range(2):
                        m = 2 * t + mm
                        dps = psum_pool.tile(
                            [P, D0_OUT], f32, name=f"dps0_{m}", tag="dense_ps",
                            bufs=2,
                        )
                        for sp in range(2):
                            nc.tensor.matmul(
                                dps[:],
                                lhsT=xtile[:, 2 * sp : 2 * sp + 2, ts(mm, P)],
                                rhs=w_sb[0][:, 2 * sp : 2 * sp + 2, :],
                                start=(sp == 0),
                                stop=(sp == 1),
                                perf_mode=DR,
                            )
                        s_sb = work.tile(
                            [P, D0_OUT], e4, name=f"ssb0_{m}", tag="s_sb0", bufs=4
                        )
                        nc.vector.tensor_scalar_mul(s_sb[:], dps[:], 1.0 / 16.0)
                        nc.scalar.dma_start(s_bounce[0][ts(m, P), :], s_sb[:])
                        if m in done_after:
                            ag_issue(0, done_after[m])
                    for _ in range(8):
                        th = next(wgen, None)
                        if th is not None:
                            th()
                for c in C_ORDER:
                    s_load(0, c)
            else:
                # layer 0: every core computes the FULL S0 = x @ W0 locally
                # (redundant) straight into s_ch8 -- no collective, so
                # startup skew is absorbed by useful work.
                wgen = deferred_weight_dmas()
                for t in range(CT // 4):
                    xtile = work.tile(
                        [P, D0_IN // P, 512], e4, name=f"xtile_{t}", tag="xtile",
                        bufs=6,
                    )
                    nc.sync.dma_start(xtile[:], xTf_d[t])
                    for mm in range(4):
                        mt = 4 * t + mm
                        dps = psum_pool.tile(
                            [P, D0_OUT], f32, name=f"dps0_{mt}", tag="dense_ps",
                            bufs=2,
                        )
                        for sp in range(2):
                            nc.tensor.matmul(
                                dps[:],
                                lhsT=xtile[:, 2 * sp : 2 * sp + 2, ts(mm, P)],
                                rhs=w_sb[0][:, 2 * sp : 2 * sp + 2, :],
                                start=(sp == 0),
                                stop=(sp == 1),
                                perf_mode=DR,
                            )
                        # gathered index: chunk 0 = tiles 0..31, 1 = 32..63,
                        # 2 = 64..79
                        cch = 0 if mt < 32 else (1 if mt < 64 else 2)
                        tt = mt - {0: 0, 1: 32, 2: 64}[cch]
                        # alternate drain engine: vector/scalar each handle
                        # half the 80 psum->e4m3 drains (either alone would
                        # rate-limit the DoubleRow dense at ~0.8us apiece)
                        if mt % 2 == 0:
                            nc.vector.tensor_scalar_mul(
                                s_ch8[cch][:, tt, :], dps[:], 1.0 / 16.0
                            )
                        else:
                            nc.scalar.activation(
                                s_ch8[cch][:, tt, :], dps[:],
                                mybir.ActivationFunctionType.Copy,
                                scale=1.0 / 16.0,
                            )
                    # trickle the W1/W2/bias resident loads through the sync
                    # queue's spare issue slots -- but only once the xtile
                    # pipeline is primed (early trickle delays packs 1-5 and
                    # stalls the dense ramp)
                    if t >= 6:
                        for _ in range(3):
                            th = next(wgen, None)
                            if th is not None:
                                th()

            # layer L spmm interleaved with layer L+1 dense + gather issue.
            for L in (0, 1):
                for nci in NCI_ORDER:
                    # on layer 0's final pass, emit layer 1's s_ch8 reloads
                    # chunk-by-chunk as their WAR hazards clear (chunk 2's
                    # producer AG hasn't issued yet -- it loads after it).
                    hook = None
                    if L == 0 and nci == NCI_ORDER[-1]:
                        hook = lambda c: s_load(1, c) if c != 2 else None
                    spmm_pass_dr(L, nci, sink_mid(L), chunk_hook=hook)
                    for m in M_OF_NC[nci]:
                        dense_m(L + 1, m)
                    ag_issue(L + 1, nci)
                    if L == 0 and nci == NCI_ORDER[-1]:
                        s_load(1, 2)
                    if L == 1:
                        # s_ch16 is untouched by layers 0/1: load layer 2's
                        # gathered chunk as soon as its AG completes.
                        s_load(2, nci)
            for nci in NCI_ORDER:
                spmm_pass_l2(nci, sink_out)

    nc.compile()
    return nc


def _get_nc():
    if "nc" not in _CACHE:
        _CACHE["nc"] = _build_bass()
    return _CACHE["nc"]


def _new_of_old():
    """old global node index -> gathered contraction index."""
    idx = np.arange(NPAD)
    k = idx // R
    r = idx % R
    return np.where(
        r < 512,
        k * 512 + r,
        np.where(
            r < 1024,
            CH_BASE[1] + k * 512 + (r - 512),
            CH_BASE[2] + k * 256 + (r - 1024),
        ),
    )


def _preprocess(x, edge_row, edge_col, edge_val, W0, W1, W2, b0, b1, b2):
    x = np.asarray(x, np.float32)
    edge_row = np.asarray(edge_row, np.int64)
    edge_col = np.asarray(edge_col, np.int64)
    edge_val = np.asarray(edge_val, np.float32)

    new_of_old = _new_of_old()

    # dense per-core adjacency blocks, transposed + permuted:
    # adjT[k][new_of_old[c], r_local] = sum of vals of edges (k*R+r_local, c)
    adjT = np.zeros((NCORES, NPAD, R), np.float32)
    core = edge_row // R
    r_local = edge_row % R
    np.add.at(adjT, (core, new_of_old[edge_col], r_local), edge_val)

    # flat blocks [20, P, KSUB, R] in contraction order C_ORDER
    def blocks_of(a):  # a: [NPAD, R] for one core
        out = []
        for c in C_ORDER:
            base, _, kt, groups = CH_BASE[c], *CHUNKS[c][1:]
            ac = a[base : base + kt * P].reshape(kt, P, R)
            for g in range(groups):
                out.append(ac[g * KSUB : (g + 1) * KSUB].transpose(1, 0, 2))
        return np.stack(out)  # [20, P, KSUB, R]

    adjA01, adjA2, adjB01, adjB2 = [], [], [], []
    for k in range(NCORES):
        blk = blocks_of(adjT[k])
        a4 = blk.astype(float8_e4m3)
        a3 = blk.astype(float8_e3m4)
        adjA01.append(np.ascontiguousarray(
            np.stack([a4[..., 0:512], a4[..., 512:1024]], axis=0)))
        adjA2.append(np.ascontiguousarray(a4[..., 1024:1280]))
        adjB01.append(np.ascontiguousarray(
            np.stack([a3[..., 0:512], a3[..., 512:1024]], axis=0)))
        adjB2.append(np.ascontiguousarray(a3[..., 1024:1280]))
    del adjT

    x_pad = np.zeros((NPAD, x.shape[1]), np.float32)
    x_pad[:N] = x

    if SHARD_L0:
        xTf_all = []
        for k in range(NCORES):
            xs = x_pad[k * R : (k + 1) * R]  # [1280, 512] plain local order
            # packs of 2 m-tiles: [5, 256 nodes, 4 c, 128 f] -> [5, f, c, n]
            xp4 = xs.reshape(5, 256, x.shape[1] // P, P)
            xTf_all.append(
                np.ascontiguousarray(xp4.transpose(0, 3, 2, 1)).astype(float8_e4m3)
            )
    else:
        old_of_new = np.empty(NPAD, np.int64)
        old_of_new[new_of_old] = np.arange(NPAD)
        # packs of 4 m-tiles: [20, 512 nodes, 4 c, 128 f] -> [20, f, c, n]
        xp4 = x_pad[old_of_new].reshape(CT // 4, 512, x.shape[1] // P, P)
        xTf = np.ascontiguousarray(xp4.transpose(0, 3, 2, 1)).astype(float8_e4m3)
        xTf_all = [xTf] * NCORES

    in_maps = []
    for k in range(NCORES):
        in_maps.append(
            {
                "xTf": xTf_all[k],
                "adjA01": adjA01[k],
                "adjA2": adjA2[k],
                "adjB01": adjB01[k],
                "adjB2": adjB2[k],
                "W0": (np.asarray(W0, np.float32) * 16.0).astype(float8_e4m3),
                "W1": np.asarray(W1, np.float32).astype(bfloat16),
                "W2": np.asarray(W2, np.float32).astype(bfloat16),
                "b0": np.asarray(b0, np.float32).reshape(-1, 1),
                "b1": np.asarray(b1, np.float32).reshape(-1, 1),
                "b2": np.asarray(b2, np.float32).reshape(-1, 1),
            }
        )
    return in_maps


def kernel(x, edge_row, edge_col, edge_val, W0, W1, W2, b0, b1, b2):
    global LAST_RESULT
    from concourse.bass_utils import run_bass_kernel_spmd

    nc = _get_nc()
    in_maps = _preprocess(
        x, edge_row, edge_col, edge_val, W0, W1, W2, b0, b1, b2
    )
    res = run_bass_kernel_spmd(
        nc,
        in_maps,
        core_ids=list(range(NCORES)),
        trace=bool(int(os.environ.get("GCN_TRACE", "0"))),
    )
    LAST_RESULT = res

    outT = np.concatenate(
        [np.asarray(res.results[k]["outT"]) for k in range(NCORES)], axis=1
    )  # [256, 10240]
    return np.ascontiguousarray(outT.T[:N]).astype(np.float32)


# revision 49
# speedup vs baseline: 1.0753x; 1.0753x over previous
"""GCN (3-layer, skip-concat) on 8 Trainium2 NeuronCores.

Strategy (hardcoded for N=10000, E=320000, dims 512/512/256):
  - Row-partition nodes across 8 cores (1280 padded rows each, N padded
    to 10240). The adjacency shard is densified on the host into
    A_k^T [10240, 1280] and stored twice in fp8: e4m3 for layers 0/1
    (whose SpMM runs fp8 DoubleRow) and e3m4 for the final layer.
    fp8 halves adjacency HBM traffic vs bf16.
  - Layer-0 dense (x @ W0) also runs DoubleRow: x, W0 in e4m3 with W0
    host-scaled by 16 (clears the e4m3 denormal floor), epilogue x1/16.
  - Layers 0/1 SpMM: S blocks (e4m3) stationary + A^T (e4m3) moving with
    perf_mode=DoubleRow -> 256-deep contraction per instruction, ~2x
    TensorE throughput. Layer 2 SpMM: e3m4 S (host-scale 1/8, sink x8)
    x e3m4 A, plain matmuls. End-to-end rel err ~1.3e-2 (verified
    numerically on the fixed seed; gate is 2e-2).
  - Activations live feature-major (X^T) in SBUF. Per layer:
      S_k   = X^T.T @ W          (node-major [1280, d_out], PE)
      S     = AllGather(S_k)     (fp8, HBM collective, 3 chunks)
      Y^T   = S_tiles^T @ A_k^T  (feature-major, PE; S tiles stationary)
      X' ^T = [relu(Y^T + b); (Y^T + b)]   (partition-axis concat, free)
  - Chunking: 3 gathered chunks per layer = local row ranges
    [0,512) / [512,1024) / [1024,1280), produced by passes nci0/1/2.
    Natural pass order (0,1,2): the first pass of each layer is a long
    512-wide one, so the last-produced chunk of the next layer (small
    ch2, consumed last in the contraction) has maximal all-gather slack.
  - A tiny warm-up AllGather at program start absorbs the CC-stream
    bootstrap (~35-110us) + initial cross-core skew, so the first real
    gather never eats them.
  - Queue discipline: sync = x packs + adjacency prefetch + deferred
    W1/W2/bias loads (trickled); scalar = epilogue drains (alternating
    with vector), bounce writes, gathered-chunk reloads; gpsimd = w0 +
    collective triggers. Layer-1's s_ch reloads are emitted mid-k-loop
    (chunk hooks) so they start the moment their WAR hazard clears.
  - SpMM PSUM tiles rotate through 5 tags so the first matmul of each
    pass never waits on the previous pass's epilogue draining the bank
    (5 spmm banks + 2 dense banks = 7 of 8).
"""

import os
import numpy as np
from ml_dtypes import bfloat16, float8_e3m4, float8_e4m3

N = 10000
NPAD = 10240
NCORES = 8
R = NPAD // NCORES  # 1280 rows per core
P = 128
CT = NPAD // P  # 80 contraction tiles for the SpMM
KSUB = 4  # contraction subtiles per adjacency DMA block

D0_IN, D0_OUT = 512, 512
D1_IN, D1_OUT = 1024, 512
D2_IN, D2_OUT = 1024, 256

# n-chunks of the 1280-wide free dim (PSUM bank = 512 fp32); pass nci
# produces gathered chunk nci (same index).
N_CHUNKS = [(0, 512), (512, 512), (1024, 256)]
# natural pass order: the FIRST pass of each layer is a long (512-wide)
# one, so the last-produced gathered chunk of the NEXT layer (small ch2,
# consumed last in the contraction) has maximal all-gather slack.
NCI_ORDER = (0, 1, 2)
# chunk c -> (local row0, nrows, gathered ktiles, KSUB-groups)
CHUNKS = {0: (0, 512, 32, 8), 1: (512, 512, 32, 8), 2: (1024, 256, 16, 4)}
C_ORDER = (0, 1, 2)  # contraction + production order
CH_BASE = {0: 0, 1: 4096, 2: 8192}  # gathered-row base of each chunk
# flat adjacency block order: (chunk, group) pairs in contraction order
BLOCKS = [(c, g) for c in C_ORDER for g in range(CHUNKS[c][3])]  # 20
# dense m-tiles whose lhsT columns come from pass nci's epilogue
M_OF_NC = {0: (0, 1, 2, 3), 1: (4, 5, 6, 7), 2: (8, 9)}

SHARD_L0 = bool(int(os.environ.get("GCN_SHARD_L0", "0")))

_CACHE = {}
LAST_RESULT = None  # BassKernelResults of the most recent run (for test.py)


def _build_bass():
    import concourse.bass as bass
    import concourse.bacc as bacc
    import concourse.mybir as mybir
    import concourse.tile as tile

    dt = mybir.dt
    bf16 = dt.bfloat16
    e4 = dt.float8e4
    e3 = dt.float8e3
    f32 = dt.float32
    ts = bass.ts
    DR = mybir.MatmulPerfMode.DoubleRow

    nc = bacc.Bacc(
        "TRN2",
        target_bir_lowering=False,
        debug=False,
        enable_asserts=False,
        num_devices=NCORES,
    )

    # x and W0 are e4m3: the layer-0 dense runs fp8 DoubleRow. W0 is
    # host-scaled by 16 (so its small values clear the e4m3 denormal
    # floor); the epilogue multiplies by 1/16. x tiles are packed 4
    # m-tiles per DMA so the dense loop isn't DMA-issue-rate bound.
    n_xp = 5 if SHARD_L0 else CT // 4
    xp_w = 256 if SHARD_L0 else 512
    xTf_d = nc.dram_tensor("xTf", [n_xp, P, D0_IN // P, xp_w], e4, kind="ExternalInput")
    # adjacency blocks [P, KSUB, nw]: e4m3 copy (layers 0/1, DoubleRow)
    # and e3m4 copy (layer 2); nci0/1 are the 512-wide column chunks.
    adjA01_d = nc.dram_tensor("adjA01", [2, 20, P, KSUB, 512], e4, kind="ExternalInput")
    adjA2_d = nc.dram_tensor("adjA2", [20, P, KSUB, 256], e4, kind="ExternalInput")
    adjB01_d = nc.dram_tensor("adjB01", [2, 20, P, KSUB, 512], e3, kind="ExternalInput")
    adjB2_d = nc.dram_tensor("adjB2", [20, P, KSUB, 256], e3, kind="ExternalInput")
    W_d = [
        nc.dram_tensor("W0", [D0_IN, D0_OUT], e4, kind="ExternalInput"),
        nc.dram_tensor("W1", [D1_IN, D1_OUT], bf16, kind="ExternalInput"),
        nc.dram_tensor("W2", [D2_IN, D2_OUT], bf16, kind="ExternalInput"),
    ]
    b_d = [
        nc.dram_tensor("b0", [D0_OUT, 1], f32, kind="ExternalInput"),
        nc.dram_tensor("b1", [D1_OUT, 1], f32, kind="ExternalInput"),
        nc.dram_tensor("b2", [D2_OUT, 1], f32, kind="ExternalInput"),
    ]
    outT_d = nc.dram_tensor("outT", [D2_OUT, R], f32, kind="ExternalOutput")

    DIMS = [(D0_IN, D0_OUT), (D1_IN, D1_OUT), (D2_IN, D2_OUT)]
    # per-layer S dtype on the gather path (stationary dtype of the
    # CONSUMING spmm): layers 0/1 e4m3 (DoubleRow), layer 2 bf16.
    S_DT = [e4, e4, e3]

    with tile.TileContext(nc) as tc:
        ctx_pools = (
            tc.tile_pool(name="persist", bufs=1),
            tc.tile_pool(name="work", bufs=3),
            tc.tile_pool(name="psum", bufs=1, space="PSUM"),
            tc.tile_pool(name="dram", bufs=1, space="DRAM"),
        )
        with ctx_pools[0] as persist, ctx_pools[1] as work, \
             ctx_pools[2] as psum_pool, ctx_pools[3] as dram_pool:

            # ---- resident weights / biases ----
            # w0 is needed immediately but issued on the gpsimd queue so the
            # sync queue's first DMAs are the xtile loads the PE waits on.
            w_sb = []
            for L, (d_in, d_out) in enumerate(DIMS):
                wt = persist.tile(
                    [P, d_in // P, d_out], e4 if L == 0 else bf16,
                    name=f"w{L}", tag=f"w{L}"
                )
                if L == 0:
                    for c in range(d_in // P):
                        nc.gpsimd.dma_start(wt[:, c, :], W_d[L][ts(c, P), :])
                w_sb.append(wt)
            warm_done = [False]

            b_sb = []
            for L, (d_in, d_out) in enumerate(DIMS):
                tiles = []
                for pi in range(d_out // P):
                    bt = persist.tile([P, 1], f32, name=f"b{L}_{pi}", tag=f"b{L}_{pi}")
                    tiles.append(bt)
                b_sb.append(tiles)

            def deferred_weight_dmas():
                """generator of thunks: W1/W2/bias loads, issued a few per
                dense iteration on the sync queue (it has spare issue slots
                during the dense phase; scalar/vector are epilogue-busy)."""
                for L in (1, 2):
                    d_in = DIMS[L][0]
                    for c in range(d_in // P):
                        yield lambda L=L, c=c: nc.sync.dma_start(
                            w_sb[L][:, c, :], W_d[L][ts(c, P), :]
                        )
                for L in range(3):
                    for pi in range(DIMS[L][1] // P):
                        yield lambda L=L, pi=pi: nc.sync.dma_start(
                            b_sb[L][pi][:], b_d[L][ts(pi, P), :]
                        )

            # ---- activations X^T (feature-major), one 3D tile per layer ----
            xt1 = persist.tile([P, D1_IN // P, R], bf16, name="xt1", tag="xt1")
            xt2 = persist.tile([P, D2_IN // P, R], bf16, name="xt2", tag="xt2")
            xts = [None, xt1, xt2]

            # gathered S: e4m3 tiles for layers 0/1 (DoubleRow stationary),
            # bf16 tiles for layer 2.
            s_ch8 = {
                c: persist.tile(
                    [P, CHUNKS[c][2], 512], e4, name=f"s8_{c}", tag=f"s8_{c}"
                )
                for c in range(3)
            }
            # layer 2's gathered S2 is e3m4, host-semantics scale 1/8
            # (values up to ~105 vs e3m4 max 15.5); sink_out multiplies by 8.
            s_ch16 = {
                c: persist.tile(
                    [P, CHUNKS[c][2], 256], e3, name=f"s16_{c}", tag=f"s16_{c}"
                )
                for c in range(3)
            }
            s_ch_of = [s_ch8, s_ch8, s_ch16]

            # tiny warm-up collective: absorbs the CC-stream bootstrap
            # (~35-110us, starts ~21us in) and initial cross-core skew so
            # the first REAL all-gather doesn't eat them. Nobody consumes
            # its output.
            cc_warm_in = dram_pool.tile([8, 8], bf16, name="ccw_in", tag="ccw_in")
            cc_warm_out = dram_pool.tile(
                [8 * NCORES, 8], bf16, name="ccw_out", tag="ccw_out",
                addr_space="Shared",
            )

            s_bounce = [
                dram_pool.tile([R, DIMS[L][1]], S_DT[L], name=f"s_bounce{L}", tag=f"sb{L}")
                for L in range(3)
            ]
            s_all = [
                {
                    c: dram_pool.tile(
                        [NCORES * CHUNKS[c][1], DIMS[L][1]],
                        S_DT[L],
                        name=f"s_all{L}_{c}",
                        tag=f"sa{L}_{c}",
                        addr_space="Shared",
                    )
                    for c in range(3)
                }
                for L in range(3)
            ]

            # rotating spmm psum tags: 5 tags over passes of <=4 tiles each
            sp_ctr = [0]

            def dense_m(L, m):
                """dense S_k m-tile: psum = xt.T @ W, cast to S dtype, bounce."""
                d_in, d_out = DIMS[L]
                n_ct = d_in // P
                dps = psum_pool.tile(
                    [P, d_out], f32, name=f"dps_{L}_{m}", tag="dense_ps", bufs=2
                )
                for c in range(n_ct):
                    nc.tensor.matmul(
                        dps[:],
                        lhsT=xts[L][:, c, ts(m, P)],
                        rhs=w_sb[L][:, c, :],
                        start=(c == 0),
                        stop=(c == n_ct - 1),
                    )
                s_sb = work.tile(
                    [P, d_out], S_DT[L], name=f"ssb_{L}_{m}", tag=f"s_sb{L}", bufs=4
                )
                # alternate the psum->fp8 drain between vector and
                # scalar so neither engine rate-limits the dense bursts
                sc = 0.125 if L == 2 else 1.0
                if m % 2 == 0:
                    if sc == 1.0:
                        nc.vector.tensor_copy(s_sb[:], dps[:])
                    else:
                        nc.vector.tensor_scalar_mul(s_sb[:], dps[:], sc)
                else:
                    nc.scalar.activation(
                        s_sb[:], dps[:], mybir.ActivationFunctionType.Copy,
                        scale=sc,
                    )
                # bounce on the scalar queue so the sync queue's adjacency
                # prefetch is never head-of-line blocked behind it
                nc.scalar.dma_start(s_bounce[L][ts(m, P), :], s_sb[:])

            def cc_warmup():
                if warm_done[0]:
                    return
                warm_done[0] = True
                nc.gpsimd.collective_compute(
                    "AllGather",
                    mybir.AluOpType.bypass,
                    replica_groups=[list(range(NCORES))],
                    ins=[cc_warm_in[:].opt()],
                    outs=[cc_warm_out[:].opt()],
                )

            def ag_issue(L, c):
                """all-gather chunk c of layer L's S (writes s_all only)."""
                r0, nrows, _, _ = CHUNKS[c]
                nc.gpsimd.collective_compute(
                    "AllGather",
                    mybir.AluOpType.bypass,
                    replica_groups=[list(range(NCORES))],
                    ins=[s_bounce[L][r0 : r0 + nrows, :].opt()],
                    outs=[s_all[L][c].opt()],
                )

            def s_load(L, c):
                """load gathered chunk c into SBUF for layer L's spmm, in
                two halves so the first k-tiles land sooner. Issued on the
                scalar queue: the sync queue's tail (adjacency prefetches,
                bounce writes) would head-of-line-block these right at the
                layer boundary."""
                d_out = DIMS[L][1]
                src = s_all[L][c].rearrange("(t p) d -> p t d", p=P)
                dst = s_ch_of[L][c]
                kt = CHUNKS[c][2]
                h = kt // 2
                nc.scalar.dma_start(dst[:, :h, :d_out], src[:, :h, :])
                nc.scalar.dma_start(dst[:, h:, :d_out], src[:, h:, :])

            def spmm_pass_dr(L, nci, sink, chunk_hook=None):
                """DoubleRow SpMM pass (layers 0/1): e4m3 x e4m3.

                chunk_hook(c) is emitted right after chunk c's last block in
                the contraction loop -- used on the layer's final pass to
                emit the next layer's s_ch reloads as early as the WAR
                hazard allows (ahead of this pass's sinks in queue order).
                """
                n_po = DIMS[L][1] // P
                n0, nw = N_CHUNKS[nci]
                sp_ps = []
                for p in range(n_po):
                    tag = sp_ctr[0] % 5
                    sp_ctr[0] += 1
                    sp_ps.append(
                        psum_pool.tile(
                            [P, nw], f32, name=f"sp_{L}_{nci}_{p}", tag=f"sp{tag}"
                        )
                    )
                first = True
                for bi, (c, g) in enumerate(BLOCKS):
                    if nci < 2:
                        at = work.tile(
                            [P, KSUB, 512], e4,
                            name=f"a4_{L}_{nci}_{bi}", tag="at4", bufs=12,
                        )
                        nc.sync.dma_start(at[:], adjA01_d[nci, bi])
                    else:
                        at = work.tile(
                            [P, KSUB, 256], e4,
                            name=f"a4n2_{L}_{bi}", tag="at4n2", bufs=5,
                        )
                        nc.sync.dma_start(at[:], adjA2_d[bi])
                    for sp in range(2):
                        last = (bi == len(BLOCKS) - 1 and sp == 1)
                        kk = g * KSUB + 2 * sp
                        for p in range(n_po):
                            nc.tensor.matmul(
                                sp_ps[p][:],
                                lhsT=s_ch8[c][:, kk : kk + 2, ts(p, P)],
                                rhs=at[:, 2 * sp : 2 * sp + 2, :],
                                start=first,
                                stop=last,
                                perf_mode=DR,
                            )
                        first = False
                    if chunk_hook is not None and g == CHUNKS[c][3] - 1:
                        chunk_hook(c)
                for p in range(n_po):
                    sink(p, sp_ps[p], n0, nw)

            def spmm_pass_l2(nci, sink):
                """Layer-2 SpMM pass: e3m4 S stationary x e3m4 A moving."""
                n_po = DIMS[2][1] // P
                n0, nw = N_CHUNKS[nci]
                sp_ps = []
                for p in range(n_po):
                    tag = sp_ctr[0] % 5
                    sp_ctr[0] += 1
                    sp_ps.append(
                        psum_pool.tile(
                            [P, nw], f32, name=f"sp_2_{nci}_{p}", tag=f"sp{tag}"
                        )
                    )
                first = True
                for bi, (c, g) in enumerate(BLOCKS):
                    if nci < 2:
                        at = work.tile(
                            [P, KSUB, 512], e3,
                            name=f"a3_{nci}_{bi}", tag="at3", bufs=10,
                        )
                        nc.sync.dma_start(at[:], adjB01_d[nci, bi])
                    else:
                        at = work.tile(
                            [P, KSUB, 256], e3,
                            name=f"a3n2_{bi}", tag="at3n2", bufs=5,
                        )
                        nc.sync.dma_start(at[:], adjB2_d[bi])
                    for s in range(KSUB):
                        last = (bi == len(BLOCKS) - 1 and s == KSUB - 1)
                        for p in range(n_po):
                            nc.tensor.matmul(
                                sp_ps[p][:],
                                lhsT=s_ch16[c][:, g * KSUB + s, ts(p, P)],
                                rhs=at[:, s, :],
                                start=first,
                                stop=last,
                            )
                        first = False
                for p in range(n_po):
                    sink(p, sp_ps[p], n0, nw)

            def sink_mid(L):
                n_po = DIMS[L][1] // P

                def sink(p, ps, n0, nw):
                    nc.scalar.activation(
                        xts[L + 1][:, p, n0 : n0 + nw],
                        ps[:],
                        mybir.ActivationFunctionType.Relu,
                        bias=b_sb[L][p][:],
                    )
                    nc.vector.tensor_scalar_add(
                        xts[L + 1][:, n_po + p, n0 : n0 + nw],
                        ps[:],
                        b_sb[L][p][:],
                    )

                return sink

            def sink_out(p, ps, n0, nw):
                ot = work.tile([P, nw], f32, name=f"ot_{n0}_{p}", tag="ot", bufs=3)
                nc.vector.tensor_scalar(
                    ot[:], ps[:], 8.0, b_sb[2][p][:],
                    mybir.AluOpType.mult, mybir.AluOpType.add,
                )
                nc.scalar.dma_start(outT_d[ts(p, P), n0 : n0 + nw], ot[:])

            # ================= pipeline =================
            cc_warmup()
            if SHARD_L0:
                # layer 0 dense: each core computes only its own 1280 rows,
                # bounced + all-gathered. m-order produces chunk 2 first.
                wgen = deferred_weight_dmas()
                done_after = {3: 0, 7: 1, 9: 2}  # m -> AG chunk complete
                for t in range(5):
                    xtile = work.tile(
                        [P, D0_IN // P, 256], e4, name=f"xtile_{t}", tag="xtile",
                        bufs=6,
                    )
                    nc.sync.dma_start(xtile[:], xTf_d[t])
                    for mm in range(2):
                        m = 2 * t + mm
                        dps = psum_pool.tile(
                            [P, D0_OUT], f32, name=f"dps0_{m}", tag="dense_ps",
                            bufs=2,
                        )
                        for sp in range(2):
                            nc.tensor.matmul(
                                dps[:],
                                lhsT=xtile[:, 2 * sp : 2 * sp + 2, ts(mm, P)],
                                rhs=w_sb[0][:, 2 * sp : 2 * sp + 2, :],
                                start=(sp == 0),
                                stop=(sp == 1),
                                perf_mode=DR,
                            )
                        s_sb = work.tile(
                            [P, D0_OUT], e4, name=f"ssb0_{m}", tag="s_sb0", bufs=4
                        )
                        nc.vector.tensor_scalar_mul(s_sb[:], dps[:], 1.0 / 16.0)
                        nc.scalar.dma_start(s_bounce[0][ts(m, P), :], s_sb[:])
                        if m in done_after:
                            ag_issue(0, done_after[m])
                    for _ in range(8):
                        th = next(wgen, None)
                        if th is not None:
                            th()
                for c in C_ORDER:
                    s_load(0, c)
            else:
                # layer 0: every core computes the FULL S0 = x @ W0 locally
                # (redundant) straight into s_ch8 -- no collective, so
                # startup skew is absorbed by useful work.
                wgen = deferred_weight_dmas()
                for t in range(CT // 4):
                    xtile = work.tile(
                        [P, D0_IN // P, 512], e4, name=f"xtile_{t}", tag="xtile",
                        bufs=6,
                    )
                    nc.sync.dma_start(xtile[:], xTf_d[t])
                    for mm in range(4):
                        mt = 4 * t + mm
                        dps = psum_pool.tile(
                            [P, D0_OUT], f32, name=f"dps0_{mt}", tag="dense_ps",
                            bufs=2,
                        )
                        for sp in range(2):
                            nc.tensor.matmul(
                                dps[:],
                                lhsT=xtile[:, 2 * sp : 2 * sp + 2, ts(mm, P)],
                                rhs=w_sb[0][:, 2 * sp : 2 * sp + 2, :],
                                start=(sp == 0),
                                stop=(sp == 1),
                                perf_mode=DR,
                            )
                        # gathered index: chunk 0 = tiles 0..31, 1 = 32..63,
                        # 2 = 64..79
                        cch = 0 if mt < 32 else (1 if mt < 64 else 2)
                        tt = mt - {0: 0, 1: 32, 2: 64}[cch]
                        # alternate drain engine: vector/scalar each handle
                        # half the 80 psum->e4m3 drains (either alone would
                        # rate-limit the DoubleRow dense at ~0.8us apiece)
                        if mt % 2 == 0:
                            nc.vector.tensor_scalar_mul(
                                s_ch8[cch][:, tt, :], dps[:], 1.0 / 16.0
                            )
                        else:
                            nc.scalar.activation(
                                s_ch8[cch][:, tt, :], dps[:],
                                mybir.ActivationFunctionType.Copy,
                                scale=1.0 / 16.0,
                            )
                    # trickle the W1/W2/bias resident loads through the sync
                    # queue's spare issue slots -- but only once the xtile
                    # pipeline is primed (early trickle delays packs 1-5 and
                    # stalls the dense ramp)
                    if t >= 6:
                        for _ in range(3):
                            th = next(wgen, None)
                            if th is not None:
                                th()

            # layer L spmm interleaved with layer L+1 dense + gather issue.
            for L in (0, 1):
                for nci in NCI_ORDER:
                    # on layer 0's final pass, emit layer 1's s_ch8 reloads
                    # chunk-by-chunk as their WAR hazards clear (chunk 2's
                    # producer AG hasn't issued yet -- it loads after it).
                    hook = None
                    if L == 0 and nci == NCI_ORDER[-1]:
                        hook = lambda c: s_load(1, c) if c != 2 else None
                    spmm_pass_dr(L, nci, sink_mid(L), chunk_hook=hook)
                    for m in M_OF_NC[nci]:
                        dense_m(L + 1, m)
                    ag_issue(L + 1, nci)
                    if L == 0 and nci == NCI_ORDER[-1]:
                        s_load(1, 2)
                    if L == 1:
                        # s_ch16 is untouched by layers 0/1: load layer 2's
                        # gathered chunk as soon as its AG completes.
                        s_load(2, nci)
            for nci in NCI_ORDER:
                spmm_pass_l2(nci, sink_out)

    nc.compile()
    return nc


def _get_nc():
    if "nc" not in _CACHE:
        _CACHE["nc"] = _build_bass()
    return _CACHE["nc"]


def _new_of_old():
    """old global node index -> gathered contraction index."""
    idx = np.arange(NPAD)
    k = idx // R
    r = idx % R
    return np.where(
        r < 512,
        k * 512 + r,
        np.where(
            r < 1024,
            CH_BASE[1] + k * 512 + (r - 512),
            CH_BASE[2] + k * 256 + (r - 1024),
        ),
    )


def _preprocess(x, edge_row, edge_col, edge_val, W0, W1, W2, b0, b1, b2):
    x = np.asarray(x, np.float32)
    edge_row = np.asarray(edge_row, np.int64)
    edge_col = np.asarray(edge_col, np.int64)
    edge_val = np.asarray(edge_val, np.float32)

    new_of_old = _new_of_old()

    # dense per-core adjacency blocks, transposed + permuted:
    # adjT[k][new_of_old[c], r_local] = sum of vals of edges (k*R+r_local, c)
    adjT = np.zeros((NCORES, NPAD, R), np.float32)
    core = edge_row // R
    r_local = edge_row % R
    np.add.at(adjT, (core, new_of_old[edge_col], r_local), edge_val)

    # flat blocks [20, P, KSUB, R] in contraction order C_ORDER
    def blocks_of(a):  # a: [NPAD, R] for one core
        out = []
        for c in C_ORDER:
            base, _, kt, groups = CH_BASE[c], *CHUNKS[c][1:]
            ac = a[base : base + kt * P].reshape(kt, P, R)
            for g in range(groups):
                out.append(ac[g * KSUB : (g + 1) * KSUB].transpose(1, 0, 2))
        return np.stack(out)  # [20, P, KSUB, R]

    adjA01, adjA2, adjB01, adjB2 = [], [], [], []
    for k in range(NCORES):
        blk = blocks_of(adjT[k])
        a4 = blk.astype(float8_e4m3)
        a3 = blk.astype(float8_e3m4)
        adjA01.append(np.ascontiguousarray(
            np.stack([a4[..., 0:512], a4[..., 512:1024]], axis=0)))
        adjA2.append(np.ascontiguousarray(a4[..., 1024:1280]))
        adjB01.append(np.ascontiguousarray(
            np.stack([a3[..., 0:512], a3[..., 512:1024]], axis=0)))
        adjB2.append(np.ascontiguousarray(a3[..., 1024:1280]))
    del adjT

    x_pad = np.zeros((NPAD, x.shape[1]), np.float32)
    x_pad[:N] = x

    if SHARD_L0:
        xTf_all = []
        for k in range(NCORES):
            xs = x_pad[k * R : (k + 1) * R]  # [1280, 512] plain local order
            # packs of 2 m-tiles: [5, 256 nodes, 4 c, 128 f] -> [5, f, c, n]
            xp4 = xs.reshape(5, 256, x.shape[1] // P, P)
            xTf_all.append(
                np.ascontiguousarray(xp4.transpose(0, 3, 2, 1)).astype(float8_e4m3)
            )
    else:
        old_of_new = np.empty(NPAD, np.int64)
        old_of_new[new_of_old] = np.arange(NPAD)
        # packs of 4 m-tiles: [20, 512 nodes, 4 c, 128 f] -> [20, f, c, n]
        xp4 = x_pad[old_of_new].reshape(CT // 4, 512, x.shape[1] // P, P)
        xTf = np.ascontiguousarray(xp4.transpose(0, 3, 2, 1)).astype(float8_e4m3)
        xTf_all = [xTf] * NCORES

    in_maps = []
    for k in range(NCORES):
        in_maps.append(
            {
                "xTf": xTf_all[k],
                "adjA01": adjA01[k],
                "adjA2": adjA2[k],
                "adjB01": adjB01[k],
                "adjB2": adjB2[k],
                "W0": (np.asarray(W0, np.float32) * 16.0).astype(float8_e4m3),
                "W1": np.asarray(W1, np.float32).astype(bfloat16),
                "W2": np.asarray(W2, np.float32).astype(bfloat16),
                "b0": np.asarray(b0, np.float32).reshape(-1, 1),
                "b1": np.asarray(b1, np.float32).reshape(-1, 1),
                "b2": np.asarray(b2, np.float32).reshape(-1, 1),
            }
        )
    return in_maps


def kernel(x, edge_row, edge_col, edge_val, W0, W1, W2, b0, b1, b2):
    global LAST_RESULT
    from concourse.bass_utils import run_bass_kernel_spmd

    nc = _get_nc()
    in_maps = _preprocess(
        x, edge_row, edge_col, edge_val, W0, W1, W2, b0, b1, b2
    )
    res = run_bass_kernel_spmd(
        nc,
        in_maps,
        core_ids=list(range(NCORES)),
        trace=bool(int(os.environ.get("GCN_TRACE", "0"))),
    )
    LAST_RESULT = res

    outT = np.concatenate(
        [np.asarray(res.results[k]["outT"]) for k in range(NCORES)], axis=1
    )  # [256, 10240]
    return np.ascontiguousarray(outT.T[:N]).astype(np.float32)
